# revision 87
# baseline (speedup 1.0000x reference)
"""Trainium2 Bass kernel for nn_LowRankSTLayer_dilation.

Mathematical reduction (validated vs the jax reference):
  1. U/V start rank-symmetric and the multiplicative NMF updates preserve
     that, so the rank-3 iteration is exactly rank-1.
  2. eps=1e-6 is negligible vs the O(1)+ denominators, so each update is a
     plain normalized projection -- power iteration on the per-position
     Gram matrix G = X X^T.  All normalization scalars cancel:
         out = relu( tail_w @ ( p3 * <h,p2>/<p3,p2> ) )
     p0 = box27(h), p_{n+1} = G p_n, G = box27(h h^T) (separable 3x3x3
     box filter applied to channel-pair products), h = relu(head_w @ x).
  3. Mixed power step: with the 120 strict pairs m_ab = box27(h_a h_b)
     and the diagonal-correction field E = 2 box27(h^2) - box27(h S)
     (S = sum_c h_c, materialized as box27 of the product h*(2h-S)),
         (G p)_a = sum_{b!=a} m_ab (p_a + p_b) + p_a E_a,
     so one PE broadcast (p_a+p_b per pair slot), one 128-row DVE mul,
     one PE reduce and one elementwise correction per application --
     half the matmul columns of the naive two-sided scheme.

Sharding: 8 cores = batch(2) x H-quarters(4); each core receives a
replicate-padded slice [17, 10, 26, 98] covering all 8 frames (+temporal
halo) for 24 output rows (channel 16 is constant 1.0).

Pipeline (bf16, PSUM fp32): three 8-row chunks per core, each
  stage1: head conv + relu + 4 selector matmuls + 2 products over the
          flattened padded chunk [17, 9800] in 20 pieces;
  box:    separable 3x3x3 box of m1 (pairs) and m2 (h / h-dup /
          E-products) in two temporal halves, engine-assigned by the
          Tile scheduler (gpsimd shares its SBUF port with the DVE and
          is ~2x slower, so in practice everything lands on the DVE);
  power:  3 applications of the mixed step over 12 pieces of 512
          positions on uniform 48-row iterates (row blocks 0-15/32-47,
          zeros at 16-31, so p2 feeds the gamma stage with no
          cross-partition moves; E is duplicated by two SBUF-to-SBUF
          DMAs); the +correction is a DVE add for apps 0-1 and a PE
          identity-accumulate + scalar copy-out for app 2 (engine
          balance);
  gamma:  num/den reduces, reciprocal, gamma broadcast, tail matmul,
          relu, DMA out.
"""

import numpy as np
import ml_dtypes
from contextlib import ExitStack

import concourse.bass as bass
import concourse.bacc as bacc
import concourse.tile as tile
from concourse import mybir
from concourse.bass_utils import run_bass_kernel_spmd

F32 = mybir.dt.float32
BF16 = mybir.dt.bfloat16

B, C, D, H, W = 2, 16, 8, 96, 96
NCORES = 8
HP, WP = H + 2, W + 2
HR = 24                           # output rows per core
DF = D + 2                        # frames incl. temporal halo
R = 8                             # output rows per chunk
NCHUNK = HR // R                  # 3
RIN = R + 2                       # input rows per chunk (10)
PIN = DF * RIN * WP               # padded positions per chunk (9800)
SP = 490                          # stage1 piece
NSP = PIN // SP                   # 20
POS = D * R * W                   # output positions per chunk (6144)
CP = 512                          # power/gamma piece
NPC = POS // CP                   # 12
FR = R * W                        # positions per frame per chunk (768)
NPAIR = 120
FH = D // 2                       # frames per box half (4)

_pairs = [(a, b) for a in range(C) for b in range(a + 1, C)]
_A = np.array([p[0] for p in _pairs])
_B = np.array([p[1] for p in _pairs])


def _build_consts(head_w, tail_w):
    hwT = head_w.T.astype(np.float32)          # [c_in, c_out]
    w_head = np.zeros((C + 1, C + 1), np.float32)
    w_head[:C, :C] = hwT
    w_head[C, C] = 1.0
    # m1 rows: 0..119 pair products h_a h_b (a<b), rows 120..127 zero
    w_a = np.zeros((C + 1, 128), np.float32)
    w_b = np.zeros((C + 1, 128), np.float32)
    w_a[_A, np.arange(NPAIR)] = 1.0
    w_b[_B, np.arange(NPAIR)] = 1.0
    # m2 rows: h at 0..15 AND 32..47 (so every power iterate lives on a
    # 48-row tile with data at row blocks 0-15/32-47 and zeros at
    # 16-31), E-products h_c*(2 h_c - S) at 48..63 where S = sum_c h_c,
    # so box27(m2[48:64]) = E = 2 box27(h^2) - box27(h S) -- the
    # diagonal-correction field of the mixed power update.
    w_f = np.zeros((C + 1, 128), np.float32)
    w_g = np.zeros((C + 1, 128), np.float32)
    for o in (0, 32):
        w_f[np.arange(C), o + np.arange(16)] = 1.0
        w_g[C, o:o + 16] = 1.0
    w_f[np.arange(C), 48 + np.arange(16)] = 1.0
    w_g[0:C, 48:64] = -1.0
    w_g[np.arange(C), 48 + np.arange(16)] = 1.0
    # mixed broadcast/reduce: slot (a,b) gets p_a + p_b, contributes to
    # both channels a and b.  p' = S_both (TA .* bcast(p)) + p .* E.
    sel_sum = np.zeros((C, 128), np.float32)
    sel_sum[_A, np.arange(NPAIR)] = 1.0
    sel_sum[_B, np.arange(NPAIR)] = 1.0
    s_both = np.zeros((128, 48), np.float32)
    for o in (0, 32):
        s_both[np.arange(NPAIR), o + _A] = 1.0
        s_both[np.arange(NPAIR), o + _B] = 1.0
    out = dict(w_head=w_head, w_a=w_a, w_b=w_b, w_f=w_f, w_g=w_g,
               sel_sum=sel_sum, s_both=s_both)
    # gamma: num/den reduces, broadcast, tail
    wnd = np.zeros((48, 2), np.float32)
    wnd[0:16, 0] = 1.0
    wnd[32:48, 1] = 1.0
    out["wnd"] = wnd
    wbc1 = np.zeros((1, 48), np.float32)
    wbc1[0, 32:48] = 1.0
    out["wbc1"] = wbc1
    tail48 = np.zeros((48, C), np.float32)
    tail48[32:48, :] = tail_w.T.astype(np.float32)
    out["tail48"] = tail48
    out["i48"] = np.eye(48, dtype=np.float32)
    return out


_CONST_SHAPES = dict(w_head=(C + 1, C + 1), w_a=(C + 1, 128),
                     w_b=(C + 1, 128), w_f=(C + 1, 128), w_g=(C + 1, 128),
                     sel_sum=(C, 128), s_both=(128, 48),
                     wnd=(48, 2), wbc1=(1, 48), tail48=(48, 16), i48=(48, 48))


def _spans(pc):
    """Decompose flat piece [CP*pc, CP*(pc+1)) into per-frame
    contiguous (frame, offset, len) spans (FR positions/frame)."""
    q, end, out = CP * pc, CP * (pc + 1), []
    while q < end:
        f, qf = divmod(q, FR)
        ln = min(FR - qf, end - q)
        out.append((f, qf, ln))
        q += ln
    return out


def _build_program():
    nc = bacc.Bacc("TRN2", target_bir_lowering=False, debug=False)
    xin = nc.declare_dram_parameter("xin", [C + 1, DF, HR + 2, WP], BF16,
                                    isOutput=False)
    cst = {k: nc.declare_dram_parameter(k, list(v), BF16, isOutput=False)
           for k, v in _CONST_SHAPES.items()}
    out = nc.declare_dram_parameter("out", [C, D, HR, W], F32, isOutput=True)

    with tile.TileContext(nc) as tc, ExitStack() as ctx:
        singles = ctx.enter_context(tc.tile_pool(name="singles", bufs=1))
        sb = {}
        for k, v in _CONST_SHAPES.items():
            sb[k] = singles.tile(list(v), BF16, tag=k, name=k)
            nc.sync.dma_start(out=sb[k], in_=cst[k][:, :])

        xpool = ctx.enter_context(tc.tile_pool(name="x", bufs=1))
        hxp = ctx.enter_context(tc.tile_pool(name="hx", bufs=2))
        ps = ctx.enter_context(tc.tile_pool(name="ps", bufs=2, space="PSUM"))
        mpool = ctx.enter_context(tc.tile_pool(name="m", bufs=1))
        boxp = ctx.enter_context(tc.tile_pool(name="box", bufs=1))
        gp = ctx.enter_context(tc.tile_pool(name="g", bufs=1))
        pp = ctx.enter_context(tc.tile_pool(name="pp", bufs=1))
        smp = ctx.enter_context(tc.tile_pool(name="sm", bufs=2))

        def s1_begin(ci):
            r0 = ci * R
            xs = xpool.tile([C + 1, DF, RIN, WP], BF16, tag="xs")
            for f in range(DF):
                nc.sync.dma_start(out=xs[:, f], in_=xin[:, f, r0:r0 + RIN, :])
            xf = xs.rearrange("c f r w -> c (f r w)")
            m1 = mpool.tile([128, PIN], BF16, tag="m1")
            m2 = mpool.tile([128, PIN], BF16, tag="m2")
            return xf, m1, m2

        def s1_piece(st, pc):
            xf, m1, m2 = st
            sl = slice(pc * SP, (pc + 1) * SP)
            ph = ps.tile([C + 1, SP], F32, tag="q0")
            nc.tensor.matmul(ph, sb["w_head"], xf[:, sl],
                             start=True, stop=True)
            hx = hxp.tile([C + 1, SP], BF16, tag="hx", bufs=2)
            nc.scalar.activation(hx, ph,
                                 mybir.ActivationFunctionType.Relu)
            pa = ps.tile([128, SP], F32, tag="q1")
            nc.tensor.matmul(pa, sb["w_a"], hx, start=True, stop=True)
            ha = smp.tile([128, SP], BF16, tag="ha", bufs=2)
            nc.scalar.copy(ha, pa)
            pb = ps.tile([128, SP], F32, tag="q2")
            nc.tensor.matmul(pb, sb["w_b"], hx, start=True, stop=True)
            nc.vector.tensor_mul(m1[:, sl], ha, pb)
            pf = ps.tile([128, SP], F32, tag="q3")
            nc.tensor.matmul(pf, sb["w_f"], hx, start=True, stop=True)
            hf = smp.tile([128, SP], BF16, tag="hf")
            nc.scalar.copy(hf, pf)
            pg = ps.tile([128, SP], F32, tag="q0")
            nc.tensor.matmul(pg, sb["w_g"], hx, start=True, stop=True)
            nc.vector.tensor_mul(m2[:, sl], hf, pg)

        def box_t(m, hf_, tagp, eng):
            """Temporal stage of the 3x3x3 box for half hf_; must be
            issued before stage1(ci+1) overwrites the m buffer."""
            v = m.rearrange("p (f r w) -> p f r w", r=RIN, w=WP)
            f0 = FH * hf_
            t0 = boxp.tile([128, FH, RIN, WP], BF16, tag=f"tA{tagp}")
            bd = boxp.tile([128, FH, RIN, WP], BF16, tag=f"tB{tagp}")
            eng.tensor_add(t0, v[:, f0:f0 + FH], v[:, f0 + 1:f0 + FH + 1])
            eng.tensor_add(bd, t0, v[:, f0 + 2:f0 + FH + 2])
            return bd

        def box_rw(bd, dst, hf_, tagp, eng):
            """Row + column stages of the box; m-independent, so these
            braid freely between stage1 pieces of the next chunk."""
            f0 = FH * hf_
            t1 = boxp.tile([128, FH, R, WP], BF16, tag=f"tA{tagp}")
            br = boxp.tile([128, FH, R, WP], BF16, tag=f"tC{tagp}")
            eng.tensor_add(t1, bd[:, :, 0:R], bd[:, :, 1:R + 1])
            eng.tensor_add(br, t1, bd[:, :, 2:R + 2])
            t2 = boxp.tile([128, FH, R, W], BF16, tag=f"tB{tagp}")
            eng.tensor_add(t2, br[:, :, :, 0:W], br[:, :, :, 2:W + 2])
            eng.tensor_add(dst[:, f0:f0 + FH], t2, br[:, :, :, 1:W + 1])

        hcp = pp.tile([48, POS], BF16, tag="hcp")
        nc.vector.memset(hcp[:, :], 0.0)
        ecg = pp.tile([48, POS], BF16, tag="ec")
        nc.vector.memset(ecg[:, :], 0.0)

        def issue_hcr(m2):
            """Centre h for gamma numerators (rows 0-15 of HCP); must be
            issued before stage1(ci+1) overwrites the m2 buffer."""
            mv = m2.rearrange("p (f r w) -> p f r w", r=RIN, w=WP)
            hcpv = hcp.rearrange("p (f r w) -> p f r w", r=R, w=W)
            nc.scalar.copy(hcpv[0:C], mv[0:C, 1:D + 1, 1:R + 1, 1:W + 1])

        ofv = out.rearrange("c f h w -> c f (h w)")

        def power_piece(pc, tav, plan, dve_add=False):
            """One 512-position piece through the mixed power
            application: p' = S_both48 (TA .* sel_sum(p)) + p .* E.
            The + is either a PE accumulate + scalar copy-out, or a DVE
            add (dve_add), to balance engine load."""
            sl = slice(pc * CP, (pc + 1) * CP)
            for pin, pout, c0 in plan:
                prs = ps.tile([128, CP], F32, tag="q0")
                nc.tensor.matmul(prs, sb["sel_sum"], pin[0:C, sl],
                                 start=True, stop=True)
                pix = gp.tile([128, CP], BF16, tag="pix", bufs=2)
                nc.vector.tensor_mul(pix, tav[:, sl], prs)
                tmp = gp.tile([48, CP], BF16, tag="tmp", bufs=2)
                nc.vector.tensor_mul(tmp, pin[:, sl], ecg[:, sl])
                if dve_add:
                    acc = ps.tile([48, CP], F32, tag="q1")
                    nc.tensor.matmul(acc, sb["s_both"], pix,
                                     start=True, stop=True)
                    nc.vector.tensor_add(pout[c0:48, sl], tmp[c0:48],
                                         acc[c0:48])
                else:
                    acc = ps.tile([48, CP], F32, tag="q1")
                    nc.tensor.matmul(acc, sb["s_both"], pix,
                                     start=True, stop=False)
                    nc.tensor.matmul(acc, sb["i48"], tmp,
                                     start=False, stop=True)
                    nc.scalar.copy(pout[c0:48, sl], acc[c0:48])

        def gamma_piece(ci, pc, pq):
            """gamma = <h,p2>/<p3,p2>; out = relu(tail (gamma*p3))."""
            sl = slice(pc * CP, (pc + 1) * CP)
            tt = gp.tile([48, CP], BF16, tag="tt", bufs=2)
            nc.vector.tensor_mul(tt, hcp[:, sl], pq[:, sl])
            pnum = ps.tile([1, CP], F32, tag="q3")
            nc.tensor.matmul(pnum, sb["wnd"][:, 0:1], tt,
                             start=True, stop=True)
            pden = ps.tile([1, CP], F32, tag="q3")
            nc.tensor.matmul(pden, sb["wnd"][:, 1:2], tt,
                             start=True, stop=True)
            rcp = gp.tile([1, CP], F32, tag="rcp", bufs=2)
            nc.vector.reciprocal_approx_fast(out=rcp, in_=pden)
            gam = gp.tile([1, CP], BF16, tag="gam", bufs=2)
            nc.vector.tensor_mul(gam, pnum, rcp)
            grep = ps.tile([48, CP], F32, tag="q0")
            nc.tensor.matmul(grep[32:48], sb["wbc1"][:, 32:48], gam,
                             start=True, stop=True)
            upre = gp.tile([48, CP], BF16, tag="upre", bufs=2)
            nc.vector.tensor_mul(upre[32:48], hcp[32:48, sl],
                                 grep[32:48])
            pout = ps.tile([C, CP], F32, tag="q1")
            nc.tensor.matmul(pout, sb["tail48"][32:48, :], upre[32:48],
                             start=True, stop=True)
            osb = smp.tile([C, CP], F32, tag="osb")
            nc.scalar.activation(osb, pout,
                                 mybir.ActivationFunctionType.Relu)
            for f, q0, ln in _spans(pc):
                nc.sync.dma_start(
                    out=ofv[:, f, ci * FR + q0:ci * FR + q0 + ln],
                    in_=osb[:, f * FR + q0 - pc * CP:
                            f * FR + q0 - pc * CP + ln])

        # Chunk loop.  Power/gamma piece-chains of chunk ci are BRAIDED
        # with stage1 pieces of chunk ci+1 so each engine FIFO always
        # holds independent work behind a stalled chain.
        st = s1_begin(0)
        for pc in range(NSP):
            s1_piece(st, pc)
        for ci in range(NCHUNK):
            _, m1, m2 = st
            ta = boxp.tile([128, D, R, W], BF16, tag="ta")
            g2f = boxp.tile([128, D, R, W], BF16, tag="g2")
            issue_hcr(m2)
            st = s1_begin(ci + 1) if ci + 1 < NCHUNK else None
            if st is None:
                for hf_ in range(2):
                    bd1 = box_t(m1, hf_, "1", nc.any)
                    bd2 = box_t(m2, hf_, "2", nc.any)
                    box_rw(bd1, ta, hf_, "1", nc.any)
                    box_rw(bd2, g2f, hf_, "2", nc.any)
            else:
                # braid: t-stages guard the m buffer, then the
                # m-independent r/w box stages interleave with stage1
                # pieces of the next chunk so the vector engine stays
                # fed while stage1's scalar-bound chain runs.
                bd1 = box_t(m1, 0, "1", nc.any)
                bd2 = box_t(m2, 0, "2", nc.any)
                for pc in range(8):
                    s1_piece(st, pc)
                    if pc == 1:
                        box_rw(bd1, ta, 0, "1", nc.any)
                    elif pc == 4:
                        box_rw(bd2, g2f, 0, "2", nc.any)
                    elif pc == 6:
                        bd1 = box_t(m1, 1, "1", nc.any)
                    elif pc == 7:
                        bd2 = box_t(m2, 1, "2", nc.any)
                for pc in range(8, NSP):
                    s1_piece(st, pc)
                    if pc == 9:
                        box_rw(bd1, ta, 1, "1", nc.any)
                    elif pc == 13:
                        box_rw(bd2, g2f, 1, "2", nc.any)
            tav = ta.rearrange("p f r w -> p (f r w)")
            g2v = g2f.rearrange("p f r w -> p (f r w)")
            # E field duplicated to row blocks 0-15/32-47 (DMA does the
            # cross-partition moves)
            nc.sync.dma_start(out=ecg[0:C, :], in_=g2v[48:64, :])
            nc.sync.dma_start(out=ecg[32:48, :], in_=g2v[48:64, :])
            p1 = pp.tile([48, POS], BF16, tag="p1")
            pq = pp.tile([48, POS], BF16, tag="pq")
            plan = [(g2v[0:48, :], p1, 0), (p1, pq, 0), (pq, hcp, 32)]
            for app in range(3):
                for pc in range(NPC):
                    power_piece(pc, tav, plan[app:app + 1],
                                dve_add=(app <= 1))
            for pc in range(NPC):
                gamma_piece(ci, pc, pq)
    nc.compile()
    return nc


_NC_CACHE = None
TRACE = False
LAST_EXEC_NS = None
LAST_RESULT = None


def kernel(x, head_w, tail_w):
    global _NC_CACHE, LAST_EXEC_NS, LAST_RESULT
    x = np.asarray(x, dtype=np.float32)
    head_w = np.asarray(head_w, dtype=np.float32)
    tail_w = np.asarray(tail_w, dtype=np.float32)

    consts = {k: v.astype(ml_dtypes.bfloat16)
              for k, v in _build_consts(head_w, tail_w).items()}
    xp = np.pad(x, ((0, 0), (0, 0), (1, 1), (1, 1), (1, 1)), mode="edge")
    in_maps = []
    for core in range(NCORES):
        b, hs = divmod(core, 4)
        xs = np.empty((C + 1, DF, HR + 2, WP), ml_dtypes.bfloat16)
        xs[:C] = xp[b, :, :, HR * hs:HR * hs + HR + 2, :]
        xs[C] = 1.0
        m = {"xin": xs}
        m.update(consts)
        in_maps.append(m)

    if _NC_CACHE is None:
        _NC_CACHE = _build_program()
    res = run_bass_kernel_spmd(_NC_CACHE, in_maps, list(range(NCORES)),
                               trace=TRACE)
    LAST_EXEC_NS = res.exec_time_ns
    LAST_RESULT = res

    outf = np.empty((B, C, D, H, W), np.float32)
    for core in range(NCORES):
        b, hs = divmod(core, 4)
        outf[b, :, :, HR * hs:HR * hs + HR] = res.results[core]["out"]
    return outf
